# revision 1
# baseline (speedup 1.0000x reference)
"""GQA kernel for Trainium2, 8 NeuronCores.

Problem: B=2, T=2048, D=2048, 16 query heads / 2 KV heads, d_head=128, causal.

Sharding: core c -> batch b = c//4, head-quarter q = c%4 (query heads
4q..4q+3, kv head q//2). Each core computes its 4 heads' attention and a
partial output projection (its Wo rows); host sums the 4 partials per batch
and adds bo.

Host marshalling: weights and x are pre-cast to bf16 (same rounding the
kernel would do on-chip) and x is supplied transposed (xT = x[b].T), which
is the layout every projection matmul consumes.

On-core dataflow (bf16 matmuls, fp32 PSUM accum), interleaved in 4 rounds
over 512-wide t-slices so PE stays continuously fed:
  round j: project KT/QT/VT for slice j; PE-transpose VT -> V natural;
           attention (h, j) for all 4 heads over tk blocks 0..4j+3
           (S_T tiles [tk,tq]; exp on ACT; causal zeroing of the diagonal
           blocks on GpSimd post-exp; OT accum on PE; row-sum accum on DVE
           with one fp32 ones-matmul per (h,j) for the partition reduction);
           output projection for the 4 t-tiles of slice j.
Engine budget: PE ~matmuls only, ACT ~exp+proj epilogues, DVE ~copies+
row-sum+normalize, GpSimd ~causal masks, 4 DMA queues for input streaming.
"""

import numpy as np
import ml_dtypes
from contextlib import ExitStack

import concourse.bass as bass
from concourse import bacc
import concourse.mybir as mybir
import concourse.tile as tile
from concourse.bass_utils import run_bass_kernel_spmd
from concourse.masks import make_identity

F32 = mybir.dt.float32
BF16 = mybir.dt.bfloat16

D = 2048
T = 2048
DH = 128
B = 2
HPC = 4            # query heads per core
NCORES = 8
SCALE = 1.0 / float(np.sqrt(128.0))

_CACHE = {}


def _build_nc():
    nc = bacc.Bacc("TRN2", target_bir_lowering=False, debug=False,
                   num_devices=NCORES)

    xt = nc.dram_tensor("xt", [D, T], BF16, kind="ExternalInput")
    wq = nc.dram_tensor("wq", [D, HPC * DH], BF16, kind="ExternalInput")
    wk = nc.dram_tensor("wk", [D, DH], BF16, kind="ExternalInput")
    wv = nc.dram_tensor("wv", [D, DH], BF16, kind="ExternalInput")
    wo = nc.dram_tensor("wo", [HPC * DH, D], BF16, kind="ExternalInput")
    bqm = nc.dram_tensor("bqm", [DH, HPC], F32, kind="ExternalInput")
    bkm = nc.dram_tensor("bkm", [DH, 1], F32, kind="ExternalInput")
    bvm = nc.dram_tensor("bvm", [DH, 1], F32, kind="ExternalInput")
    part = nc.dram_tensor("part", [T, D], F32, kind="ExternalOutput")

    with ExitStack() as ctx:
        tc = ctx.enter_context(tile.TileContext(nc))
        persist = ctx.enter_context(tc.tile_pool(name="persist", bufs=1))
        work = ctx.enter_context(tc.tile_pool(name="work", bufs=3))
        psum = ctx.enter_context(tc.tile_pool(name="psum", bufs=2, space="PSUM"))

        # ---- constants ----
        ones32 = persist.tile([128, 128], F32, tag="ones32", name="ones32")
        nc.vector.memset(ones32, 1.0)
        ident = persist.tile([128, 128], BF16, tag="ident", name="ident")
        make_identity(nc, ident)

        bq_sb = persist.tile([DH, HPC], F32, tag="bq", name="bq_sb")
        nc.sync.dma_start(out=bq_sb, in_=bqm[:, :])
        bk_sb = persist.tile([DH, 1], F32, tag="bk", name="bk_sb")
        nc.sync.dma_start(out=bk_sb, in_=bkm[:, :])
        bv_sb = persist.tile([DH, 1], F32, tag="bv", name="bv_sb")
        nc.sync.dma_start(out=bv_sb, in_=bvm[:, :])

        # ---- inputs -> SBUF (already bf16), streamed on 4 DMA queues in
        # consumption order: wk, xT slice 0, wq, wv, xT slices 1-3, wo ----
        queues = [nc.sync, nc.scalar, nc.gpsimd]
        _qi = [0]

        def dma(out, in_):
            queues[_qi[0] % 3].dma_start(out=out, in_=in_)
            _qi[0] += 1

        xT = [persist.tile([128, T], BF16, tag=f"xT{kb}", name=f"xT{kb}")
              for kb in range(16)]
        wq_sb = []
        wk_sb = []
        wv_sb = []
        for kb in range(16):
            wkt = persist.tile([128, 128], BF16, tag=f"wk{kb}", name=f"wk_sb{kb}")
            dma(wkt, wk[kb * 128:(kb + 1) * 128, :])
            wk_sb.append(wkt)
        for kb in range(16):
            dma(xT[kb][:, 0:512], xt[kb * 128:(kb + 1) * 128, 0:512])
        for kb in range(16):
            wqt = persist.tile([128, 512], BF16, tag=f"wq{kb}", name=f"wq_sb{kb}")
            dma(wqt, wq[kb * 128:(kb + 1) * 128, :])
            wq_sb.append(wqt)
        for kb in range(16):
            wvt = persist.tile([128, 128], BF16, tag=f"wv{kb}", name=f"wv_sb{kb}")
            dma(wvt, wv[kb * 128:(kb + 1) * 128, :])
            wv_sb.append(wvt)
        for js in range(1, 4):
            for kb in range(16):
                dma(xT[kb][:, js * 512:(js + 1) * 512],
                    xt[kb * 128:(kb + 1) * 128, js * 512:(js + 1) * 512])
        wo_sb = []
        for h in range(HPC):
            wot = persist.tile([128, D], BF16, tag=f"wo{h}", name=f"wo_sb{h}")
            dma(wot, wo[h * 128:(h + 1) * 128, :])
            wo_sb.append(wot)

        # ---- persistent activations ----
        qT = [persist.tile([128, T], BF16, tag=f"qT{h}", name=f"qT{h}")
              for h in range(HPC)]
        kT = persist.tile([128, T], BF16, tag="kT", name="kT")
        v_sb = [persist.tile([128, DH], BF16, tag=f"v{t}", name=f"v{t}")
                for t in range(16)]
        oT = [persist.tile([128, T], BF16, tag=f"oT{h}", name=f"oT{h}")
              for h in range(HPC)]

        for j in range(4):
            sl = slice(j * 512, (j + 1) * 512)

            # --- projections for t-slice j ---
            kps = psum.tile([128, 512], F32, tag="acc", bufs=3, name=f"kps{j}")
            for kb in range(16):
                nc.tensor.matmul(out=kps, lhsT=wk_sb[kb], rhs=xT[kb][:, sl],
                                 start=(kb == 0), stop=(kb == 15))
            nc.scalar.activation(out=kT[:, sl], in_=kps,
                                 func=mybir.ActivationFunctionType.Identity,
                                 bias=bk_sb[:, 0:1], scale=1.0)

            for h in range(HPC):
                qps = psum.tile([128, 512], F32, tag="acc", bufs=3,
                                name=f"qps{j}_{h}")
                for kb in range(16):
                    nc.tensor.matmul(out=qps,
                                     lhsT=wq_sb[kb][:, h * 128:(h + 1) * 128],
                                     rhs=xT[kb][:, sl],
                                     start=(kb == 0), stop=(kb == 15))
                nc.scalar.activation(out=qT[h][:, sl], in_=qps,
                                     func=mybir.ActivationFunctionType.Identity,
                                     bias=bq_sb[:, h:h + 1], scale=1.0)

            # VT projection for slice j, then PE-transpose to natural V
            vps = psum.tile([128, 512], F32, tag="acc", bufs=3, name=f"vps{j}")
            for kb in range(16):
                nc.tensor.matmul(out=vps, lhsT=wv_sb[kb], rhs=xT[kb][:, sl],
                                 start=(kb == 0), stop=(kb == 15))
            vt_sb = work.tile([128, 512], BF16, tag="vt", bufs=2,
                              name=f"vt{j}")
            nc.scalar.activation(out=vt_sb, in_=vps,
                                 func=mybir.ActivationFunctionType.Identity,
                                 bias=bv_sb[:, 0:1], scale=1.0)
            vtp = psum.tile([128, 512], BF16, tag="op", bufs=2, name=f"vtp{j}")
            for sub in range(4):
                nc.tensor.transpose(vtp[:, sub * 128:(sub + 1) * 128],
                                    vt_sb[:, sub * 128:(sub + 1) * 128],
                                    ident)
            for sub in range(4):
                nc.vector.tensor_copy(out=v_sb[4 * j + sub],
                                      in_=vtp[:, sub * 128:(sub + 1) * 128])

            # --- attention for all heads, tq-slice j ---
            ntk = 4 * (j + 1)
            for h in range(HPC):
                otps = psum.tile([128, 512], F32, tag="acc", bufs=3,
                                 name=f"otps{h}_{j}")
                racc = work.tile([128, 512], F32, tag="racc", bufs=2,
                                 name=f"racc{h}_{j}")
                for tkb in range(ntk):
                    sps = psum.tile([128, 512], F32, tag="sp", bufs=3,
                                    name=f"sps{h}_{j}_{tkb}")
                    nc.tensor.matmul(out=sps,
                                     lhsT=kT[:, tkb * 128:(tkb + 1) * 128],
                                     rhs=qT[h][:, sl],
                                     start=True, stop=True)
                    pt = work.tile([128, 512], BF16, tag="pt", bufs=6,
                                   name=f"pt{h}_{j}_{tkb}")
                    nc.scalar.activation(out=pt, in_=sps,
                                         func=mybir.ActivationFunctionType.Exp,
                                         scale=SCALE)
                    if tkb >= 4 * j:
                        # causal: zero pt where tq_free < tk_part + 128*r
                        nc.gpsimd.affine_select(
                            out=pt, in_=pt,
                            compare_op=mybir.AluOpType.is_ge,
                            fill=0.0,
                            base=-(128 * (tkb - 4 * j)),
                            pattern=[[1, 512]],
                            channel_multiplier=-1,
                        )
                    nc.tensor.matmul(out=otps, lhsT=v_sb[tkb], rhs=pt,
                                     start=(tkb == 0), stop=(tkb == ntk - 1))
                    if tkb == 0:
                        nc.vector.tensor_copy(out=racc, in_=pt)
                    else:
                        nc.vector.tensor_add(out=racc, in0=racc, in1=pt)
                rsb = psum.tile([128, 512], F32, tag="acc", bufs=3,
                                name=f"rsb{h}_{j}")
                nc.tensor.matmul(out=rsb, lhsT=ones32, rhs=racc,
                                 start=True, stop=True)
                rinv = work.tile([128, 512], F32, tag="rinv", bufs=2,
                                 name=f"rinv{h}_{j}")
                nc.vector.reciprocal_approx_fast(rinv, rsb)
                nc.vector.tensor_mul(out=oT[h][:, sl], in0=otps, in1=rinv)

            # --- output projection for the 4 t-tiles of slice j ---
            for sub in range(4):
                tt = 4 * j + sub
                ostg = work.tile([128, D], F32, tag="ostg", bufs=2,
                                 name=f"ostg{tt}")
                for n in range(4):
                    ops = psum.tile([128, 512], F32, tag="op", bufs=2,
                                    name=f"ops{tt}_{n}")
                    for h in range(HPC):
                        nc.tensor.matmul(
                            out=ops,
                            lhsT=oT[h][:, tt * 128:(tt + 1) * 128],
                            rhs=wo_sb[h][:, n * 512:(n + 1) * 512],
                            start=(h == 0), stop=(h == HPC - 1))
                    nc.vector.tensor_copy(out=ostg[:, n * 512:(n + 1) * 512],
                                          in_=ops)
                nc.sync.dma_start(out=part[tt * 128:(tt + 1) * 128, :],
                                  in_=ostg)

    nc.compile()
    return nc


def _get_nc():
    if "nc" not in _CACHE:
        _CACHE["nc"] = _build_nc()
    return _CACHE["nc"]


def _bf16(a):
    return np.ascontiguousarray(a.astype(ml_dtypes.bfloat16))


def kernel(x, Wq, bq, Wk, bk, Wv, bv, Wo, bo, **kw):
    x = np.asarray(x, dtype=np.float32)
    Wq = np.asarray(Wq, dtype=np.float32)
    Wk = np.asarray(Wk, dtype=np.float32)
    Wv = np.asarray(Wv, dtype=np.float32)
    Wo = np.asarray(Wo, dtype=np.float32)
    bq = np.asarray(bq, dtype=np.float32)
    bk = np.asarray(bk, dtype=np.float32)
    bv = np.asarray(bv, dtype=np.float32)
    bo = np.asarray(bo, dtype=np.float32)

    nc = _get_nc()
    xt_b = [_bf16(x[b].T) for b in range(B)]
    in_maps = []
    for c in range(NCORES):
        b = c // 4
        q = c % 4
        hs = q * HPC * DH          # column start in Wq / row start in Wo
        kv = q // 2
        bq_m = np.ascontiguousarray(
            bq[hs:hs + HPC * DH].reshape(HPC, DH).T)          # [128, 4]
        bk_m = np.ascontiguousarray(
            bk[kv * DH:(kv + 1) * DH].reshape(DH, 1))         # [128, 1]
        bv_m = np.ascontiguousarray(
            bv[kv * DH:(kv + 1) * DH].reshape(DH, 1))         # [128, 1]
        in_maps.append({
            "xt": xt_b[b],
            "wq": _bf16(Wq[:, hs:hs + HPC * DH]),
            "wk": _bf16(Wk[:, kv * DH:(kv + 1) * DH]),
            "wv": _bf16(Wv[:, kv * DH:(kv + 1) * DH]),
            "wo": _bf16(Wo[hs:hs + HPC * DH, :]),
            "bqm": bq_m,
            "bkm": bk_m,
            "bvm": bv_m,
        })

    res = run_bass_kernel_spmd(nc, in_maps, list(range(NCORES)),
                               **kw.get("_run_kwargs", {}))
    if kw.get("_return_res"):
        return res
    parts = [res.results[c]["part"] for c in range(NCORES)]
    out = np.empty((B, T, D), dtype=np.float32)
    for b in range(B):
        acc = parts[4 * b].astype(np.float32).copy()
        for q in range(1, 4):
            acc += parts[4 * b + q]
        out[b] = acc + bo[None, :]
    return out



# revision 5
# speedup vs baseline: 1.4944x; 1.4944x over previous
"""GQA kernel for Trainium2, 8 NeuronCores.

Problem: B=2, T=2048, D=2048, 16 query heads / 2 KV heads, d_head=128, causal.

Sharding: core c -> batch b = c//4, head-quarter q = c%4 (query heads
4q..4q+3, kv head q//2). Each core computes its 4 heads' attention and a
partial output projection (its Wo rows); host sums the 4 partials per batch
and adds bo. Partials are written bf16 (halves output DMA; host sums f32).

On-core dataflow (bf16 matmuls, fp32 PSUM accum), 4 rounds over 512-wide
tq-slices. Per round j: Q proj for slice j upfront (plus K proj round 0);
then per head an S phase: S^T tiles [tk,tq] -> exp (ACT) -> causal mask of
the 128-wide boundary block (GpSimd) -> row-sum accumulation in a bf16 racc
(DVE, 2-byte fast path). Because ACT exp (~620ns/tile) is slower than the
two PE matmuls per tile (~430ns), S matmuls are interleaved with "filler"
PE work pulled from a FIFO of thunks: K proj (j>0), V proj + PE transpose,
the previous round's output projection, the previous head's PV chain, and
row-sum matmuls (ones_bf16 @ racc_bf16, 1 cyc/row). Diagonal-band tiles are
trimmed to their causal width (512-128r), shrinking S/PV/exp/add/mask work.
This keeps the PE array continuously busy (p-state stays at max clock).
"""

import numpy as np
import ml_dtypes
from contextlib import ExitStack
from collections import deque

import concourse.bass as bass
from concourse import bacc
import concourse.mybir as mybir
import concourse.tile as tile
from concourse.bass_utils import run_bass_kernel_spmd
from concourse.masks import make_identity

F32 = mybir.dt.float32
BF16 = mybir.dt.bfloat16
IDENT = mybir.ActivationFunctionType.Identity
EXP = mybir.ActivationFunctionType.Exp

D = 2048
T = 2048
DH = 128
B = 2
HPC = 4            # query heads per core
NCORES = 8
SCALE = 1.0 / float(np.sqrt(128.0))

_CACHE = {}


def _build_nc():
    nc = bacc.Bacc("TRN2", target_bir_lowering=False, debug=False,
                   num_devices=NCORES)

    xt = nc.dram_tensor("xt", [D, T], BF16, kind="ExternalInput")
    wq = nc.dram_tensor("wq", [D, HPC * DH], BF16, kind="ExternalInput")
    wk = nc.dram_tensor("wk", [D, DH], BF16, kind="ExternalInput")
    wv = nc.dram_tensor("wv", [D, DH], BF16, kind="ExternalInput")
    wo = nc.dram_tensor("wo", [HPC * DH, D], BF16, kind="ExternalInput")
    bqm = nc.dram_tensor("bqm", [DH, HPC], F32, kind="ExternalInput")
    bkm = nc.dram_tensor("bkm", [DH, 1], F32, kind="ExternalInput")
    bvm = nc.dram_tensor("bvm", [DH, 1], F32, kind="ExternalInput")
    part = nc.dram_tensor("part", [T, D], BF16, kind="ExternalOutput")

    with ExitStack() as ctx:
        tc = ctx.enter_context(tile.TileContext(nc))
        persist = ctx.enter_context(tc.tile_pool(name="persist", bufs=1))
        work = ctx.enter_context(tc.tile_pool(name="work", bufs=3))
        psum = ctx.enter_context(tc.tile_pool(name="psum", bufs=2, space="PSUM"))

        # ---- SBUF destinations for inputs ----
        wk_sb = [persist.tile([128, 128], BF16, tag=f"wk{kb}", name=f"wk{kb}")
                 for kb in range(16)]
        wq_sb = [persist.tile([128, 512], BF16, tag=f"wq{kb}", name=f"wq{kb}")
                 for kb in range(16)]
        wv_sb = [persist.tile([128, 128], BF16, tag=f"wv{kb}", name=f"wv{kb}")
                 for kb in range(16)]
        wo_sb = [persist.tile([128, D], BF16, tag=f"wo{h}", name=f"wo{h}")
                 for h in range(HPC)]
        xT = [persist.tile([128, T], BF16, tag=f"xT{kb}", name=f"xT{kb}")
              for kb in range(16)]
        bq_sb = persist.tile([DH, HPC], F32, tag="bq", name="bq_sb")
        bk_sb = persist.tile([DH, 1], F32, tag="bk", name="bk_sb")
        bv_sb = persist.tile([DH, 1], F32, tag="bv", name="bv_sb")

        # ---- input DMAs first, in consumption order ----
        # phase A (wk, wq, xT slice0) round-robin over 4 idle-at-start queues;
        # phase B (wv, wo, xT slices 1-3 as one wide transfer per kb) on sync.
        qa = [nc.sync, nc.scalar, nc.gpsimd]
        _qi = [0]

        def dma_a(out, in_):
            qa[_qi[0] % 3].dma_start(out=out, in_=in_)
            _qi[0] += 1

        for kb in range(16):
            dma_a(wk_sb[kb], wk[kb * 128:(kb + 1) * 128, :])
        for kb in range(16):
            dma_a(wq_sb[kb], wq[kb * 128:(kb + 1) * 128, :])
        for kb in range(16):
            dma_a(xT[kb][:, 0:512], xt[kb * 128:(kb + 1) * 128, 0:512])
        nc.gpsimd.dma_start(out=bq_sb, in_=bqm[:, :])
        nc.gpsimd.dma_start(out=bk_sb, in_=bkm[:, :])
        nc.gpsimd.dma_start(out=bv_sb, in_=bvm[:, :])
        for kb in range(16):
            nc.sync.dma_start(out=wv_sb[kb], in_=wv[kb * 128:(kb + 1) * 128, :])
        for h in range(HPC):
            nc.sync.dma_start(out=wo_sb[h], in_=wo[h * 128:(h + 1) * 128, :])
        for kb in range(16):
            nc.sync.dma_start(out=xT[kb][:, 512:2048],
                              in_=xt[kb * 128:(kb + 1) * 128, 512:2048])

        # ---- constants ----
        ones_bf = persist.tile([128, 128], BF16, tag="ones", name="ones_bf")
        nc.vector.memset(ones_bf, 1.0)
        ident = persist.tile([128, 128], BF16, tag="ident", name="ident")
        make_identity(nc, ident)

        # ---- persistent activations ----
        qT = [persist.tile([128, T], BF16, tag=f"qT{h}", name=f"qT{h}")
              for h in range(HPC)]
        kT = persist.tile([128, T], BF16, tag="kT", name="kT")
        v_sb = [persist.tile([128, DH], BF16, tag=f"v{t}", name=f"v{t}")
                for t in range(16)]
        oT = [persist.tile([128, T], BF16, tag=f"oT{h}", name=f"oT{h}")
              for h in range(HPC)]

        # ---- filler machinery: FIFO of (kind, fn) emitting one PE op each ----
        filler = deque()

        def pull(n):
            for _ in range(n):
                if not filler:
                    return
                filler.popleft()[1]()

        def drain(kinds=None):
            while filler and (kinds is None or filler[0][0] in kinds):
                filler.popleft()[1]()

        def push_K(j):
            sl = slice(512 * j, 512 * (j + 1))
            st = {}

            def mk(kb):
                def f():
                    if kb == 0:
                        st['p'] = psum.tile([128, 512], F32, tag="fill",
                                            bufs=2, name=f"kps{j}")
                    nc.tensor.matmul(out=st['p'], lhsT=wk_sb[kb],
                                     rhs=xT[kb][:, sl],
                                     start=(kb == 0), stop=(kb == 15))
                    if kb == 15:
                        nc.scalar.activation(out=kT[:, sl], in_=st['p'],
                                             func=IDENT, bias=bk_sb[:, 0:1],
                                             scale=1.0)
                return f
            for kb in range(16):
                filler.append(('K', mk(kb)))

        def push_V(j):
            sl = slice(512 * j, 512 * (j + 1))
            st = {}

            def mk(kb):
                def f():
                    if kb == 0:
                        st['p'] = psum.tile([128, 512], F32, tag="fill",
                                            bufs=2, name=f"vps{j}")
                    nc.tensor.matmul(out=st['p'], lhsT=wv_sb[kb],
                                     rhs=xT[kb][:, sl],
                                     start=(kb == 0), stop=(kb == 15))
                    if kb == 15:
                        st['vt'] = work.tile([128, 512], BF16, tag="vt",
                                             bufs=2, name=f"vt{j}")
                        nc.scalar.activation(out=st['vt'], in_=st['p'],
                                             func=IDENT, bias=bv_sb[:, 0:1],
                                             scale=1.0)
                return f
            for kb in range(16):
                filler.append(('V', mk(kb)))

            def mt(sub):
                def f():
                    if sub == 0:
                        st['tp'] = psum.tile([128, 512], BF16, tag="fill",
                                             bufs=2, name=f"vtp{j}")
                    nc.tensor.transpose(st['tp'][:, sub * 128:(sub + 1) * 128],
                                        st['vt'][:, sub * 128:(sub + 1) * 128],
                                        ident)
                    nc.vector.tensor_copy(
                        out=v_sb[4 * j + sub],
                        in_=st['tp'][:, sub * 128:(sub + 1) * 128])
                return f
            for sub in range(4):
                filler.append(('V', mt(sub)))

        def push_O(j):
            # output projection for the 4 t-tiles of tq-slice j
            for sub in range(4):
                tt = 4 * j + sub
                st = {}

                def mk(n, h, tt=tt, st=st):
                    def f():
                        if h == 0:
                            if n == 0:
                                st['g'] = work.tile([128, D], BF16, tag="ostg",
                                                    bufs=2, name=f"ostg{tt}")
                            st['p'] = psum.tile([128, 512], F32, tag="fill",
                                                bufs=2, name=f"ops{tt}_{n}")
                        nc.tensor.matmul(
                            out=st['p'],
                            lhsT=oT[h][:, tt * 128:(tt + 1) * 128],
                            rhs=wo_sb[h][:, n * 512:(n + 1) * 512],
                            start=(h == 0), stop=(h == HPC - 1))
                        if h == HPC - 1:
                            nsl = slice(n * 512, (n + 1) * 512)
                            if n % 2 == 0:
                                nc.scalar.activation(out=st['g'][:, nsl],
                                                     in_=st['p'], func=IDENT,
                                                     bias=0.0, scale=1.0)
                            else:
                                nc.vector.tensor_copy(out=st['g'][:, nsl],
                                                      in_=st['p'])
                            if n == 3:
                                nc.sync.dma_start(
                                    out=part[tt * 128:(tt + 1) * 128, :],
                                    in_=st['g'])
                    return f
                for n in range(4):
                    for h in range(HPC):
                        filler.append(('O', mk(n, h)))

        def push_PV(h, j, pts, racc):
            ntk = 4 * (j + 1)
            st = {}

            def mk(tkb):
                r = tkb - 4 * j
                off = 128 * r if r > 0 else 0

                def f():
                    if tkb == 0:
                        st['p'] = psum.tile([128, 512], F32, tag="att",
                                            bufs=3, name=f"otps{h}_{j}")
                    nc.tensor.matmul(out=st['p'][:, off:512],
                                     lhsT=v_sb[tkb], rhs=pts[tkb][:, off:512],
                                     start=(tkb == 0), stop=(tkb == ntk - 1))
                return f
            for tkb in range(ntk):
                filler.append(('PV', mk(tkb)))

            def rs():
                rsb = psum.tile([128, 512], F32, tag="att", bufs=3,
                                name=f"rsb{h}_{j}")
                nc.tensor.matmul(out=rsb, lhsT=ones_bf, rhs=racc,
                                 start=True, stop=True)
                rinv = work.tile([128, 512], F32, tag="rinv", bufs=2,
                                 name=f"rinv{h}_{j}")
                nc.vector.reciprocal_approx_fast(rinv, rsb)
                nc.vector.tensor_mul(out=oT[h][:, 512 * j:512 * (j + 1)],
                                     in0=st['p'], in1=rinv)
            filler.append(('RS', rs))

        # ================= main schedule =================
        for j in range(4):
            sl = slice(512 * j, 512 * (j + 1))

            # finish all queued filler (incl. last head's PV + rowsum)
            # before the long Q-proj block; PE runs it back-to-back
            drain()

            # --- Q proj for slice j (and K proj round 0) upfront,
            #     chains interleaved per kb so they stream behind DMA ---
            qps = []
            for h in range(HPC):
                tag = "sp" if h < 3 else "fill"
                qps.append(psum.tile([128, 512], F32, tag=tag,
                                     bufs=(3 if h < 3 else 2),
                                     name=f"qps{j}_{h}"))
            kps0 = None
            if j == 0:
                kps0 = psum.tile([128, 512], F32, tag="fill", bufs=2,
                                 name="kps0")
            for kb in range(16):
                if kps0 is not None:
                    nc.tensor.matmul(out=kps0, lhsT=wk_sb[kb],
                                     rhs=xT[kb][:, sl],
                                     start=(kb == 0), stop=(kb == 15))
                for h in range(HPC):
                    nc.tensor.matmul(out=qps[h],
                                     lhsT=wq_sb[kb][:, h * 128:(h + 1) * 128],
                                     rhs=xT[kb][:, sl],
                                     start=(kb == 0), stop=(kb == 15))
            if kps0 is not None:
                nc.scalar.activation(out=kT[:, sl], in_=kps0, func=IDENT,
                                     bias=bk_sb[:, 0:1], scale=1.0)
            for h in range(HPC):
                nc.scalar.activation(out=qT[h][:, sl], in_=qps[h], func=IDENT,
                                     bias=bq_sb[:, h:h + 1], scale=1.0)

            # --- queue this round's filler ---
            if j > 0:
                push_K(j)
            push_V(j)
            if j > 0:
                push_O(j - 1)

            # --- attention S phases ---
            ntk = 4 * (j + 1)
            for h in range(HPC):
                racc = work.tile([128, 512], BF16, tag="racc", bufs=3,
                                 name=f"racc{h}_{j}")
                pts = []
                for tkb in range(ntk):
                    r = tkb - 4 * j
                    if r == 0 and h == 0:
                        # diagonal tiles need kT(j): force K chain out first
                        drain(kinds=('K',))
                    off = 128 * r if r > 0 else 0
                    sps = psum.tile([128, 512], F32, tag="sp", bufs=3,
                                    name=f"sps{h}_{j}_{tkb}")
                    nc.tensor.matmul(
                        out=sps[:, off:512],
                        lhsT=kT[:, tkb * 128:(tkb + 1) * 128],
                        rhs=qT[h][:, 512 * j + off:512 * (j + 1)],
                        start=True, stop=True)
                    pt = work.tile([128, 512], BF16, tag="pt", bufs=24,
                                   name=f"pt{h}_{j}_{tkb}")
                    nc.scalar.activation(out=pt[:, off:512],
                                         in_=sps[:, off:512],
                                         func=EXP, scale=SCALE)
                    if r >= 0:
                        nc.gpsimd.affine_select(
                            out=pt[:, off:off + 128], in_=pt[:, off:off + 128],
                            compare_op=mybir.AluOpType.is_ge,
                            fill=0.0, base=0,
                            pattern=[[1, 128]],
                            channel_multiplier=-1)
                    if tkb == 0:
                        nc.vector.tensor_copy(out=racc, in_=pt)
                    else:
                        nc.vector.tensor_add(out=racc[:, off:512],
                                             in0=racc[:, off:512],
                                             in1=pt[:, off:512])
                    pts.append(pt)
                    pull(2 if r < 0 else 1)
                push_PV(h, j, pts, racc)

        # tail: previous-head PV/rowsum + O proj of slice 3
        drain()
        push_O(3)
        drain()

    nc.compile()
    return nc


def _get_nc():
    if "nc" not in _CACHE:
        _CACHE["nc"] = _build_nc()
    return _CACHE["nc"]


def _bf16(a):
    return np.ascontiguousarray(a.astype(ml_dtypes.bfloat16))


def kernel(x, Wq, bq, Wk, bk, Wv, bv, Wo, bo, **kw):
    x = np.asarray(x, dtype=np.float32)
    Wq = np.asarray(Wq, dtype=np.float32)
    Wk = np.asarray(Wk, dtype=np.float32)
    Wv = np.asarray(Wv, dtype=np.float32)
    Wo = np.asarray(Wo, dtype=np.float32)
    bq = np.asarray(bq, dtype=np.float32)
    bk = np.asarray(bk, dtype=np.float32)
    bv = np.asarray(bv, dtype=np.float32)
    bo = np.asarray(bo, dtype=np.float32)

    nc = _get_nc()
    xt_b = [_bf16(x[b].T) for b in range(B)]
    in_maps = []
    for c in range(NCORES):
        b = c // 4
        q = c % 4
        hs = q * HPC * DH          # column start in Wq / row start in Wo
        kv = q // 2
        bq_m = np.ascontiguousarray(
            bq[hs:hs + HPC * DH].reshape(HPC, DH).T)          # [128, 4]
        bk_m = np.ascontiguousarray(
            bk[kv * DH:(kv + 1) * DH].reshape(DH, 1))         # [128, 1]
        bv_m = np.ascontiguousarray(
            bv[kv * DH:(kv + 1) * DH].reshape(DH, 1))         # [128, 1]
        in_maps.append({
            "xt": xt_b[b],
            "wq": _bf16(Wq[:, hs:hs + HPC * DH]),
            "wk": _bf16(Wk[:, kv * DH:(kv + 1) * DH]),
            "wv": _bf16(Wv[:, kv * DH:(kv + 1) * DH]),
            "wo": _bf16(Wo[hs:hs + HPC * DH, :]),
            "bqm": bq_m,
            "bkm": bk_m,
            "bvm": bv_m,
        })

    res = run_bass_kernel_spmd(nc, in_maps, list(range(NCORES)),
                               **kw.get("_run_kwargs", {}))
    if kw.get("_return_res"):
        return res
    parts = [res.results[c]["part"] for c in range(NCORES)]
    out = np.empty((B, T, D), dtype=np.float32)
    for b in range(B):
        acc = parts[4 * b].astype(np.float32)
        for q in range(1, 4):
            acc = acc + parts[4 * b + q].astype(np.float32)
        out[b] = acc + bo[None, :]
    return out


# revision 11
# speedup vs baseline: 1.5166x; 1.0149x over previous
"""GQA kernel for Trainium2, 8 NeuronCores.

Problem: B=2, T=2048, D=2048, 16 query heads / 2 KV heads, d_head=128, causal.

Sharding: core c -> batch b = c//4, head-quarter q = c%4 (query heads
4q..4q+3, kv head q//2). Each core computes its 4 heads' attention and a
partial output projection (its Wo rows); host sums the 4 partials per batch
and adds bo. Partials are written bf16 (halves output DMA; host sums f32).

On-core dataflow (bf16 matmuls, fp32 PSUM accum), 4 rounds over 512-wide
tq-slices. Per round j: Q proj for slice j upfront (plus K proj round 0);
then per head an S phase: S^T tiles [tk,tq] -> exp (ACT) -> causal mask of
the 128-wide boundary block (GpSimd) -> row-sum accumulation in a bf16 racc
(DVE, 2-byte fast path). Because ACT exp (~620ns/tile) is slower than the
two PE matmuls per tile (~430ns), S matmuls are interleaved with "filler"
PE work pulled from a FIFO of thunks: K proj (j>0), V proj + PE transpose,
the previous round's output projection, the previous head's PV chain, and
row-sum matmuls (ones_bf16 @ racc_bf16, 1 cyc/row). Diagonal-band tiles are
trimmed to their causal width (512-128r), shrinking S/PV/exp/add/mask work.
This keeps the PE array continuously busy (p-state stays at max clock).
"""

import numpy as np
import ml_dtypes
from contextlib import ExitStack
from collections import deque

import concourse.bass as bass
from concourse import bacc
import concourse.mybir as mybir
import concourse.tile as tile
from concourse.bass_utils import run_bass_kernel_spmd
from concourse.masks import make_identity

F32 = mybir.dt.float32
BF16 = mybir.dt.bfloat16
IDENT = mybir.ActivationFunctionType.Identity
EXP = mybir.ActivationFunctionType.Exp

D = 2048
T = 2048
DH = 128
B = 2
HPC = 4            # query heads per core
NCORES = 8
SCALE = 1.0 / float(np.sqrt(128.0))

_CACHE = {}


def _build_nc():
    nc = bacc.Bacc("TRN2", target_bir_lowering=False, debug=False,
                   num_devices=NCORES)

    # host-marshalled layouts: partition-major [p, block, cols] flattened
    xt0 = nc.dram_tensor("xt0", [128, 16 * 512], BF16, kind="ExternalInput")
    xtr = nc.dram_tensor("xtr", [128, 16 * 1536], BF16, kind="ExternalInput")
    wq = nc.dram_tensor("wq", [128, HPC * 16 * 128], BF16, kind="ExternalInput")
    wk = nc.dram_tensor("wk", [128, 16 * 128], BF16, kind="ExternalInput")
    wv = nc.dram_tensor("wv", [128, 16 * 128], BF16, kind="ExternalInput")
    wo = nc.dram_tensor("wo", [128, HPC * D], BF16, kind="ExternalInput")
    bqm = nc.dram_tensor("bqm", [DH, HPC], F32, kind="ExternalInput")
    bkm = nc.dram_tensor("bkm", [DH, 1], F32, kind="ExternalInput")
    bvm = nc.dram_tensor("bvm", [DH, 1], F32, kind="ExternalInput")
    part = nc.dram_tensor("part", [T, D], BF16, kind="ExternalOutput")

    with ExitStack() as ctx:
        tc = ctx.enter_context(tile.TileContext(nc))
        persist = ctx.enter_context(tc.tile_pool(name="persist", bufs=1))
        work = ctx.enter_context(tc.tile_pool(name="work", bufs=3))
        psum = ctx.enter_context(tc.tile_pool(name="psum", bufs=2, space="PSUM"))

        # ---- SBUF destinations for inputs (big consolidated tiles) ----
        wk_sb = persist.tile([128, 16 * 128], BF16, tag="wk", name="wk_sb")
        wq_sb = persist.tile([128, HPC * 16 * 128], BF16, tag="wq",
                             name="wq_sb")
        wv_sb = persist.tile([128, 16 * 128], BF16, tag="wv", name="wv_sb")
        wo_sb = persist.tile([128, HPC * D], BF16, tag="wo", name="wo_sb")
        x0_sb = persist.tile([128, 16 * 512], BF16, tag="x0", name="x0_sb")
        xr_sb = persist.tile([128, 16 * 1536], BF16, tag="xr", name="xr_sb")
        bq_sb = persist.tile([DH, HPC], F32, tag="bq", name="bq_sb")
        bk_sb = persist.tile([DH, 1], F32, tag="bk", name="bk_sb")
        bv_sb = persist.tile([DH, 1], F32, tag="bv", name="bv_sb")

        def wkap(kb):
            return wk_sb[:, kb * 128:(kb + 1) * 128]

        def wvap(kb):
            return wv_sb[:, kb * 128:(kb + 1) * 128]

        def wqap(h, kb):
            o = h * 2048 + kb * 128
            return wq_sb[:, o:o + 128]

        def woap(h, nsl):
            return wo_sb[:, h * 2048 + nsl.start:h * 2048 + nsl.stop]

        def xap(kb, j):
            if j == 0:
                return x0_sb[:, kb * 512:(kb + 1) * 512]
            o = kb * 1536 + (j - 1) * 512
            return xr_sb[:, o:o + 512]

        # ---- input DMAs first, in consumption order, few big transfers ----
        nc.sync.dma_start(out=wk_sb, in_=wk[:, :])
        nc.scalar.dma_start(out=wq_sb[:, 0:2048], in_=wq[:, 0:2048])
        for g in range(4):
            nc.sync.dma_start(out=x0_sb[:, g * 2048:(g + 1) * 2048],
                              in_=xt0[:, g * 2048:(g + 1) * 2048])
        for h in range(1, HPC):
            nc.scalar.dma_start(out=wq_sb[:, h * 2048:(h + 1) * 2048],
                                in_=wq[:, h * 2048:(h + 1) * 2048])
        nc.gpsimd.dma_start(out=bq_sb, in_=bqm[:, :])
        nc.gpsimd.dma_start(out=bk_sb, in_=bkm[:, :])
        nc.gpsimd.dma_start(out=bv_sb, in_=bvm[:, :])
        nc.scalar.dma_start(out=wv_sb, in_=wv[:, :])
        nc.scalar.dma_start(out=wo_sb, in_=wo[:, :])
        for g in range(4):
            nc.sync.dma_start(out=xr_sb[:, g * 6144:(g + 1) * 6144],
                              in_=xtr[:, g * 6144:(g + 1) * 6144])

        # ---- constants ----
        ones_bf = persist.tile([128, 128], BF16, tag="ones", name="ones_bf")
        nc.vector.memset(ones_bf, 1.0)
        ident = persist.tile([128, 128], BF16, tag="ident", name="ident")
        make_identity(nc, ident)

        # ---- persistent activations ----
        qT = [persist.tile([128, T], BF16, tag=f"qT{h}", name=f"qT{h}")
              for h in range(HPC)]
        kT = persist.tile([128, T], BF16, tag="kT", name="kT")
        v_sb = [persist.tile([128, DH], BF16, tag=f"v{t}", name=f"v{t}")
                for t in range(16)]
        oT = [persist.tile([128, T], BF16, tag=f"oT{h}", name=f"oT{h}")
              for h in range(HPC)]

        # ---- filler machinery: FIFO of (kind, fn) emitting one PE op each ----
        filler = deque()

        def pull(n):
            for _ in range(n):
                if not filler:
                    return
                filler.popleft()[1]()

        def drain(kinds=None):
            while filler and (kinds is None or filler[0][0] in kinds):
                filler.popleft()[1]()

        def push_K(j):
            sl = slice(512 * j, 512 * (j + 1))
            st = {}

            def mk(kb):
                def f():
                    if kb == 0:
                        st['p'] = psum.tile([128, 512], F32, tag="fill",
                                            bufs=2, name=f"kps{j}")
                    nc.tensor.matmul(out=st['p'], lhsT=wkap(kb),
                                     rhs=xap(kb, j),
                                     start=(kb == 0), stop=(kb == 15))
                    if kb == 15:
                        nc.scalar.activation(out=kT[:, sl], in_=st['p'],
                                             func=IDENT, bias=bk_sb[:, 0:1],
                                             scale=1.0)
                return f
            for kb in range(16):
                filler.append(('K', mk(kb)))

        def push_V(j):
            st = {}

            def mk(kb):
                def f():
                    if kb == 0:
                        st['p'] = psum.tile([128, 512], F32, tag="fill",
                                            bufs=2, name=f"vps{j}")
                    nc.tensor.matmul(out=st['p'], lhsT=wvap(kb),
                                     rhs=xap(kb, j),
                                     start=(kb == 0), stop=(kb == 15))
                    if kb == 15:
                        st['vt'] = work.tile([128, 512], BF16, tag="vt",
                                             bufs=2, name=f"vt{j}")
                        nc.scalar.activation(out=st['vt'], in_=st['p'],
                                             func=IDENT, bias=bv_sb[:, 0:1],
                                             scale=1.0)
                return f
            for kb in range(16):
                filler.append(('V', mk(kb)))

            def mt(sub):
                def f():
                    if sub == 0:
                        st['tp'] = psum.tile([128, 512], BF16, tag="fill",
                                             bufs=2, name=f"vtp{j}")
                    nc.tensor.transpose(st['tp'][:, sub * 128:(sub + 1) * 128],
                                        st['vt'][:, sub * 128:(sub + 1) * 128],
                                        ident)
                    nc.vector.tensor_copy(
                        out=v_sb[4 * j + sub],
                        in_=st['tp'][:, sub * 128:(sub + 1) * 128])
                return f
            for sub in range(4):
                filler.append(('V', mt(sub)))

        def push_O(j):
            # output projection for the 4 t-tiles of tq-slice j
            for sub in range(4):
                tt = 4 * j + sub
                st = {}

                def mk(n, h, tt=tt, st=st):
                    def f():
                        if h == 0:
                            if n == 0:
                                st['g'] = work.tile([128, D], BF16, tag="ostg",
                                                    bufs=2, name=f"ostg{tt}")
                            st['p'] = psum.tile([128, 512], F32, tag="fill",
                                                bufs=2, name=f"ops{tt}_{n}")
                        nsl = slice(n * 512, (n + 1) * 512)
                        nc.tensor.matmul(
                            out=st['p'],
                            lhsT=oT[h][:, tt * 128:(tt + 1) * 128],
                            rhs=woap(h, nsl),
                            start=(h == 0), stop=(h == HPC - 1))
                        if h == HPC - 1:
                            if n % 2 == 0:
                                nc.scalar.activation(out=st['g'][:, nsl],
                                                     in_=st['p'], func=IDENT,
                                                     bias=0.0, scale=1.0)
                            else:
                                nc.vector.tensor_copy(out=st['g'][:, nsl],
                                                      in_=st['p'])
                            nc.sync.dma_start(
                                out=part[tt * 128:(tt + 1) * 128, nsl],
                                in_=st['g'][:, nsl])
                    return f
                for n in range(4):
                    for h in range(HPC):
                        filler.append(('O', mk(n, h)))

        def push_PV(h, j, pts, racc):
            ntk = 4 * (j + 1)
            st = {}

            def mk(tkb):
                r = tkb - 4 * j
                off = 128 * r if r > 0 else 0

                def f():
                    if tkb == 0:
                        st['p'] = psum.tile([128, 512], F32, tag="att",
                                            bufs=3, name=f"otps{h}_{j}")
                    nc.tensor.matmul(out=st['p'][:, off:512],
                                     lhsT=v_sb[tkb], rhs=pts[tkb][:, off:512],
                                     start=(tkb == 0), stop=(tkb == ntk - 1))
                return f
            for tkb in range(ntk):
                filler.append(('PV', mk(tkb)))

            def rs():
                rsb = psum.tile([128, 512], F32, tag="att", bufs=3,
                                name=f"rsb{h}_{j}")
                nc.tensor.matmul(out=rsb, lhsT=ones_bf, rhs=racc,
                                 start=True, stop=True)
                rinv = work.tile([128, 512], F32, tag="rinv", bufs=2,
                                 name=f"rinv{h}_{j}")
                nc.vector.reciprocal_approx_fast(rinv, rsb)
                nc.vector.tensor_mul(out=oT[h][:, 512 * j:512 * (j + 1)],
                                     in0=st['p'], in1=rinv)
            filler.append(('RS', rs))

        # ================= main schedule =================
        for j in range(4):
            sl = slice(512 * j, 512 * (j + 1))

            # finish all queued filler (incl. last head's PV + rowsum)
            # before the long Q-proj block; PE runs it back-to-back
            drain()

            # --- Q proj for slice j (and K proj round 0) upfront,
            #     chains interleaved per kb so they stream behind DMA ---
            qps = []
            for h in range(HPC):
                tag = "sp" if h < 3 else "fill"
                qps.append(psum.tile([128, 512], F32, tag=tag,
                                     bufs=(3 if h < 3 else 2),
                                     name=f"qps{j}_{h}"))
            kps0 = None
            if j == 0:
                kps0 = psum.tile([128, 512], F32, tag="fill", bufs=2,
                                 name="kps0")
            for kb in range(16):
                if kps0 is not None:
                    nc.tensor.matmul(out=kps0, lhsT=wkap(kb),
                                     rhs=xap(kb, j),
                                     start=(kb == 0), stop=(kb == 15))
                for h in range(HPC):
                    nc.tensor.matmul(out=qps[h],
                                     lhsT=wqap(h, kb),
                                     rhs=xap(kb, j),
                                     start=(kb == 0), stop=(kb == 15))
            if kps0 is not None:
                nc.scalar.activation(out=kT[:, sl], in_=kps0, func=IDENT,
                                     bias=bk_sb[:, 0:1], scale=1.0)
            for h in range(HPC):
                nc.scalar.activation(out=qT[h][:, sl], in_=qps[h], func=IDENT,
                                     bias=bq_sb[:, h:h + 1], scale=1.0)

            # --- queue this round's filler ---
            if j > 0:
                push_K(j)
            push_V(j)
            if j > 0:
                push_O(j - 1)

            # --- attention S phases ---
            ntk = 4 * (j + 1)
            for h in range(HPC):
                racc = work.tile([128, 512], BF16, tag="racc", bufs=3,
                                 name=f"racc{h}_{j}")
                pts = []
                for tkb in range(ntk):
                    r = tkb - 4 * j
                    if r == 0 and h == 0:
                        # diagonal tiles need kT(j): force K chain out first
                        drain(kinds=('K',))
                    off = 128 * r if r > 0 else 0
                    sps = psum.tile([128, 512], F32, tag="sp", bufs=3,
                                    name=f"sps{h}_{j}_{tkb}")
                    nc.tensor.matmul(
                        out=sps[:, off:512],
                        lhsT=kT[:, tkb * 128:(tkb + 1) * 128],
                        rhs=qT[h][:, 512 * j + off:512 * (j + 1)],
                        start=True, stop=True)
                    pt = work.tile([128, 512], BF16, tag="pt", bufs=24,
                                   name=f"pt{h}_{j}_{tkb}")
                    nc.scalar.activation(out=pt[:, off:512],
                                         in_=sps[:, off:512],
                                         func=EXP, scale=SCALE)
                    if r >= 0:
                        nc.gpsimd.affine_select(
                            out=pt[:, off:off + 128], in_=pt[:, off:off + 128],
                            compare_op=mybir.AluOpType.is_ge,
                            fill=0.0, base=0,
                            pattern=[[1, 128]],
                            channel_multiplier=-1)
                    if tkb == 0:
                        nc.vector.tensor_copy(out=racc, in_=pt)
                    else:
                        nc.vector.tensor_add(out=racc[:, off:512],
                                             in0=racc[:, off:512],
                                             in1=pt[:, off:512])
                    pts.append(pt)
                    pull(2 if r < 0 else 1)
                push_PV(h, j, pts, racc)

        # tail: previous-head PV/rowsum + O proj of slice 3
        drain()
        push_O(3)
        drain()

    nc.compile()
    return nc


def _get_nc():
    if "nc" not in _CACHE:
        _CACHE["nc"] = _build_nc()
    return _CACHE["nc"]


def _bf16(a):
    return np.ascontiguousarray(a.astype(ml_dtypes.bfloat16))


def kernel(x, Wq, bq, Wk, bk, Wv, bv, Wo, bo, **kw):
    x = np.asarray(x, dtype=np.float32)
    Wq = np.asarray(Wq, dtype=np.float32)
    Wk = np.asarray(Wk, dtype=np.float32)
    Wv = np.asarray(Wv, dtype=np.float32)
    Wo = np.asarray(Wo, dtype=np.float32)
    bq = np.asarray(bq, dtype=np.float32)
    bk = np.asarray(bk, dtype=np.float32)
    bv = np.asarray(bv, dtype=np.float32)
    bo = np.asarray(bo, dtype=np.float32)

    nc = _get_nc()

    def pmaj(a, nblk, cols):
        # [nblk*128, cols] -> partition-major [128, nblk*cols]
        return np.ascontiguousarray(
            a.reshape(nblk, 128, cols).transpose(1, 0, 2).reshape(
                128, nblk * cols))

    xt_b = []
    for b in range(B):
        xb = x[b].T                                 # [D, T]
        xt_b.append((_bf16(pmaj(xb[:, 0:512], 16, 512)),
                     _bf16(pmaj(xb[:, 512:2048], 16, 1536))))
    kv_cache = {}
    for kv in (0, 1):
        kv_cache[kv] = (_bf16(pmaj(Wk[:, kv * DH:(kv + 1) * DH], 16, DH)),
                        _bf16(pmaj(Wv[:, kv * DH:(kv + 1) * DH], 16, DH)))
    in_maps = []
    for c in range(NCORES):
        b = c // 4
        q = c % 4
        hs = q * HPC * DH          # column start in Wq / row start in Wo
        kv = q // 2
        # wq: [p, h, kb, 128] layout
        wq_m = np.ascontiguousarray(
            Wq[:, hs:hs + HPC * DH].reshape(16, 128, HPC, DH)
            .transpose(1, 2, 0, 3).reshape(128, HPC * 16 * DH))
        wo_m = pmaj(Wo[hs:hs + HPC * DH, :], HPC, D)
        bq_m = np.ascontiguousarray(
            bq[hs:hs + HPC * DH].reshape(HPC, DH).T)          # [128, 4]
        bk_m = np.ascontiguousarray(
            bk[kv * DH:(kv + 1) * DH].reshape(DH, 1))         # [128, 1]
        bv_m = np.ascontiguousarray(
            bv[kv * DH:(kv + 1) * DH].reshape(DH, 1))         # [128, 1]
        in_maps.append({
            "xt0": xt_b[b][0],
            "xtr": xt_b[b][1],
            "wq": _bf16(wq_m),
            "wk": kv_cache[kv][0],
            "wv": kv_cache[kv][1],
            "wo": _bf16(wo_m),
            "bqm": bq_m,
            "bkm": bk_m,
            "bvm": bv_m,
        })

    res = run_bass_kernel_spmd(nc, in_maps, list(range(NCORES)),
                               **kw.get("_run_kwargs", {}))
    if kw.get("_return_res"):
        return res
    parts = [res.results[c]["part"] for c in range(NCORES)]
    out = np.empty((B, T, D), dtype=np.float32)
    for b in range(B):
        acc = parts[4 * b].astype(np.float32)
        for q in range(1, 4):
            acc = acc + parts[4 * b + q].astype(np.float32)
        out[b] = acc + bo[None, :]
    return out
